# revision 9
# baseline (speedup 1.0000x reference)
"""Trainium2 Bass kernel for nn_EvoSNN (2-layer leaky-integrate-and-fire SNN).

V3 strategy (8 NeuronCores, data-parallel over batch, 256 rows per core):
  This is a memory-regime problem: the binding resource is DMA of x
  (fp32 = 80MB/core = 224us at 358GB/s). V3 ships x in 3 bytes/elem
  instead of 4 -> DMA roofline ~168us/core:
    xh  = fp16(x)                   [2B]  (packed [112, 50*7*512])
    xl8 = fp8e4m3((x - xh) * 2048)  [1B]  (same packing)
  Accuracy comes from a 3-stream matmul decomposition (CPU-simulated
  rel err 4.1e-3 vs the 2e-2 gate):
    psA = xh @ wh16                 7 fp16 matmuls   (512 cyc each)
    psB = xl8 @ wh8  (x residual)   3 DoubleRow fp8 pairs + 1 regular
        + x8  @ wls8 (w residual)   3 DoubleRow fp8 pairs + 1 regular
    cur = psA + psB / 2048
  where x8 = fp8e4(xh) is converted ON DEVICE (scalar engine chunks 0-3,
  gpsimd chunks 4-6; both HW-probed exact) and wls8 = fp8e4((w1-wh16)*2048)
  carries the fp16 weight-rounding correction. DoubleRow contracts TWO
  112-row chunks per instruction at 0.5 cyc/row; the chunked [112,7,512]
  tile layout is already a valid [Ki,Ko=2,dim] DR operand (stride 512%16==0;
  stationaries are padded to 112 cols so stride 112%16==0). HW probe:
  3xDR+1 group matches fp64 ref to 8.7e-5.
  PE per tile: 7*512 + 2*(3*256+512) ~= 6100 cyc -> ~130us/core + layer2,
  under the 168us DMA roof; vector work (converts ~65us/engine + LIF DVE
  ~65us) also hides under DMA.
  Phase 2 (sequential LIF on DVE) and layer 2 (w2 fp32r matmul with -I
  fold for the spk2 reset) are inherited from V2, as is the LAG/SKEW
  software pipelining of phase-1 tiles vs phase-2 steps.
  V2 (4B exact fp16 limbs, 21 fp16 matmuls) measured 303us; V3 targets
  ~175-195us (DMA-bound).
"""

import sys

for _p in ("/opt/trn_rl_repo", "/root/.axon_site/_ro/trn_rl_repo"):
    if _p not in sys.path:
        sys.path.append(_p)

import numpy as np

T, B, IN, HID, OUT = 100, 2048, 784, 100, 10
NCORES = 8
BS = B // NCORES          # 256 batch rows per core
TB = T * BS               # 25600
TILE = 512                # tb columns per phase-1 tile (2 timesteps)
ITERS = TB // TILE        # 50
NCH = 7                   # K chunks of 112 over IN=784
CH = IN // NCH            # 112
MP = 112                  # padded stationary cols for fp8 DR (112%16==0)
LAG = 1
SKEW = 2                  # layer-1 runs SKEW steps ahead of layer-2
REPEAT = 1
SC = 2048.0               # residual limb scale
X_BUFS = 3
X8_BUFS = 3
PSA_BUFS = 3
PSB_BUFS = 3
PM2_BUFS = 2
SCALAR_CHUNKS = 4         # chunks 0..3 converted on scalar engine
VARIANT = "full"          # full | dma | p1 | nodr | noconv  (ablation)

_cache = {}


def _build():
    import concourse.bacc as bacc
    import concourse.mybir as mybir
    from concourse.tile import TileContext

    F32 = mybir.dt.float32
    F32R = mybir.dt.float32r
    F16 = mybir.dt.float16
    F8E4 = mybir.dt.float8e4
    AO = mybir.AluOpType
    DR = mybir.MatmulPerfMode.DoubleRow
    COPY = mybir.ActivationFunctionType.Copy

    nc = bacc.Bacc("TRN2", target_bir_lowering=False, debug=False)
    xh = nc.dram_tensor("xh", [CH, ITERS * NCH * TILE], F16,
                        kind="ExternalInput").ap()
    xl8 = nc.dram_tensor("xl8", [CH, ITERS * NCH * TILE], F8E4,
                         kind="ExternalInput").ap()
    wht = nc.dram_tensor("wht", [CH, NCH * HID], F16,
                         kind="ExternalInput").ap()
    wh8p = nc.dram_tensor("wh8p", [CH, NCH * MP], F8E4,
                          kind="ExternalInput").ap()
    wls8p = nc.dram_tensor("wls8p", [CH, NCH * MP], F8E4,
                           kind="ExternalInput").ap()
    w2a = nc.dram_tensor("w2a", [HID, OUT], F32, kind="ExternalInput").ap()
    out = nc.dram_tensor("out", [OUT, BS], F32, kind="ExternalOutput").ap()

    with TileContext(nc) as tc:
        with (
            tc.tile_pool(name="const", bufs=1) as constp,
            tc.tile_pool(name="xt", bufs=X_BUFS) as xtp,
            tc.tile_pool(name="x8", bufs=X8_BUFS) as x8p,
            tc.tile_pool(name="st", bufs=1) as stp,
            tc.tile_pool(name="psa", bufs=PSA_BUFS, space="PSUM") as psap,
            tc.tile_pool(name="psb", bufs=PSB_BUFS, space="PSUM") as psbp,
            tc.tile_pool(name="pm2", bufs=PM2_BUFS, space="PSUM") as pm2p,
        ):
            # ---------------- weights ----------------
            wh = constp.tile([CH, NCH, HID], F16, tag="wh")
            nc.sync.dma_start(wh[:], wht.rearrange("p (c h) -> p c h", c=NCH))
            wh8 = constp.tile([CH, NCH, MP], F8E4, tag="wh8")
            nc.sync.dma_start(
                wh8[:], wh8p.rearrange("p (c h) -> p c h", c=NCH)
            )
            wls8 = constp.tile([CH, NCH, MP], F8E4, tag="wls8")
            nc.sync.dma_start(
                wls8[:], wls8p.rearrange("p (c h) -> p c h", c=NCH)
            )

            w2f = constp.tile([HID, OUT], F32, tag="w2f")
            nc.sync.dma_start(w2f[:], w2a)
            w2r = constp.tile([HID, OUT], F32R, tag="w2r")
            nc.sync.dma_start(w2r[:], w2f[:].bitcast(F32R))

            # ---------------- state ----------------
            mem1 = stp.tile([HID, BS], F32, tag="mem1")
            mem2 = stp.tile([OUT, BS], F32, tag="mem2")
            acc = stp.tile([OUT, BS], F32, tag="acc")
            nbuf = SKEW + 1
            spk1_tiles = []
            for k in range(nbuf):
                spk1_k = stp.tile([HID, BS], F32R, tag=f"spk1_{k}",
                                  name=f"spk1_{k}")
                spk1_tiles.append(spk1_k)
            spk2t = stp.tile([OUT, BS], F32R, tag="spk2")
            nc.gpsimd.memset(mem1[:], 0.0)
            nc.gpsimd.memset(mem2[:], 0.0)
            nc.gpsimd.memset(acc[:], 0.0)
            for k in range(nbuf):
                nc.gpsimd.memset(spk1_tiles[k][:].bitcast(F32), 0.0)
            nc.gpsimd.memset(spk2t[:].bitcast(F32), 0.0)
            # spk1 rotating buffers: step t writes spk1_bufs[t % (SKEW+1)];
            # layer-1 runs SKEW steps ahead of layer-2, so layer-2's matmul
            # still sees step t's spikes after later layer-1 steps completed
            spk1_bufs = [t[:] for t in spk1_tiles]
            spk2 = spk2t[:]
            spk2_f = spk2.bitcast(F32)
            # negI [OUT, OUT] f32r: folds "- spk2_prev" into the p2 PSUM group
            negI_f = constp.tile([OUT, OUT], F32, tag="negIf")
            nc.gpsimd.memset(negI_f[:], 0.0)
            nc.gpsimd.affine_select(
                out=negI_f[:], in_=negI_f[:], compare_op=AO.not_equal,
                fill=-1.0, base=0, pattern=[[-1, OUT]], channel_multiplier=1,
            )
            negI = constp.tile([OUT, OUT], F32R, tag="negIr")
            nc.sync.dma_start(negI[:], negI_f[:].bitcast(F32R))

            cur_tiles = []

            def phase1_iter(i):
                # packed layout: one 7KB-contiguous read per partition/tile
                span = NCH * TILE
                xh_t = xtp.tile([CH, NCH, TILE], F16, tag="xh")
                nc.sync.dma_start(
                    xh_t[:],
                    xh[:, span * i : span * (i + 1)].rearrange(
                        "p (c n) -> p c n", c=NCH
                    ),
                )
                xl_t = xtp.tile([CH, NCH, TILE], F8E4, tag="xl")
                nc.sync.dma_start(
                    xl_t[:],
                    xl8[:, span * i : span * (i + 1)].rearrange(
                        "p (c n) -> p c n", c=NCH
                    ),
                )
                if VARIANT == "dma":
                    return
                # x8 = fp8e4(xh), split across scalar + gpsimd engines
                if VARIANT not in ("noconv", "p1noconv"):
                    x8_t = x8p.tile([CH, NCH, TILE], F8E4, tag="x8")
                    nc.scalar.activation(
                        out=x8_t[:, 0:SCALAR_CHUNKS, :],
                        in_=xh_t[:, 0:SCALAR_CHUNKS, :], func=COPY,
                    )
                    nc.gpsimd.tensor_copy(
                        out=x8_t[:, SCALAR_CHUNKS:NCH, :],
                        in_=xh_t[:, SCALAR_CHUNKS:NCH, :],
                    )
                else:
                    x8_t = xl_t  # timing-only stand-in
                # main stream: 7 fp16 matmuls
                psA = psap.tile([HID, TILE], F32, tag="psa")
                for c in range(NCH):
                    nc.tensor.matmul(
                        psA[:], wh[:, c, :], xh_t[:, c, :],
                        start=(c == 0), stop=(c == NCH - 1),
                    )
                if VARIANT == "p1main":
                    cur_tiles.append((psA, psA))
                    return
                # correction stream: x-res (xl8 @ wh8) + w-res (x8 @ wls8),
                # 3 DoubleRow pairs + 1 regular each, one PSUM group
                psB = psbp.tile([MP, TILE], F32, tag="psb")
                if VARIANT in ("nodr", "p1nodr"):
                    for c in range(NCH):
                        nc.tensor.matmul(
                            psB[0:HID, :], wh8[:, c, 0:HID], xl_t[:, c, :],
                            start=(c == 0), stop=False,
                        )
                    for c in range(NCH):
                        nc.tensor.matmul(
                            psB[0:HID, :], wls8[:, c, 0:HID], x8_t[:, c, :],
                            start=False, stop=(c == NCH - 1),
                        )
                else:
                    for j in range(3):
                        nc.tensor.matmul(
                            psB[:], wh8[:, 2 * j : 2 * j + 2, :],
                            xl_t[:, 2 * j : 2 * j + 2, :],
                            start=(j == 0), stop=False, perf_mode=DR,
                        )
                    nc.tensor.matmul(
                        psB[0:HID, :], wh8[:, NCH - 1, 0:HID],
                        xl_t[:, NCH - 1, :], start=False, stop=False,
                    )
                    for j in range(3):
                        nc.tensor.matmul(
                            psB[:], wls8[:, 2 * j : 2 * j + 2, :],
                            x8_t[:, 2 * j : 2 * j + 2, :],
                            start=False, stop=False, perf_mode=DR,
                        )
                    nc.tensor.matmul(
                        psB[0:HID, :], wls8[:, NCH - 1, 0:HID],
                        x8_t[:, NCH - 1, :], start=False, stop=True,
                    )
                cur_tiles.append((psA, psB))

            def layer1_step(t):
                i, half = divmod(t, 2)
                sl = slice(BS * half, BS * (half + 1))
                psA, psB = cur_tiles[i]
                nc.vector.scalar_tensor_tensor(
                    out=mem1[:], in0=mem1[:], scalar=0.9, in1=psA[:, sl],
                    op0=AO.mult, op1=AO.add,
                )
                nc.vector.scalar_tensor_tensor(
                    out=mem1[:], in0=psB[0:HID, sl], scalar=1.0 / SC,
                    in1=mem1[:], op0=AO.mult, op1=AO.add,
                )
                nc.vector.tensor_tensor(
                    out=mem1[:], in0=mem1[:],
                    in1=spk1_bufs[(t - 1) % nbuf].bitcast(F32), op=AO.subtract
                )
                nc.vector.tensor_scalar(
                    out=spk1_bufs[t % nbuf], in0=mem1[:], scalar1=1.0,
                    scalar2=None, op0=AO.is_gt,
                )

            def layer2_step(t):
                p2 = pm2p.tile([OUT, BS], F32, tag="p2")
                nc.tensor.matmul(
                    p2[:], w2r[:], spk1_bufs[t % nbuf], start=True, stop=False
                )
                nc.tensor.matmul(p2[:], negI[:], spk2, start=False, stop=True)
                nc.vector.scalar_tensor_tensor(
                    out=mem2[:], in0=mem2[:], scalar=0.9, in1=p2[:],
                    op0=AO.mult, op1=AO.add,
                )
                nc.vector.tensor_scalar(
                    out=spk2, in0=mem2[:], scalar1=1.0, scalar2=None,
                    op0=AO.is_gt,
                )
                nc.gpsimd.tensor_tensor(
                    out=acc[:], in0=acc[:], in1=spk2_f, op=AO.add
                )

            def phase2_pair(ta, tb_):
                # layer-1 runs SKEW steps ahead of layer-2 so the DVE never
                # waits on layer-2's PE matmul (its sem round-trip hides
                # behind SKEW steps of layer-1 work).
                for t in (ta, tb_):
                    if t < T:
                        layer1_step(t)
                    if 0 <= t - SKEW < T:
                        layer2_step(t - SKEW)

            run_p2 = VARIANT not in (
                "dma", "p1", "p1nodr", "p1noconv", "p1main"
            )
            for _rep in range(REPEAT):
                cur_tiles.clear()
                for i in range(ITERS):
                    if run_p2 and i >= LAG:
                        phase2_pair(2 * (i - LAG), 2 * (i - LAG) + 1)
                    phase1_iter(i)
                if run_p2:
                    t0 = 2 * (ITERS - LAG)
                    for t in range(t0, T + SKEW + 1, 2):
                        phase2_pair(t, t + 1)

            nc.sync.dma_start(out, acc[:])

    nc.compile()
    return nc


def _get_nc():
    if "nc" not in _cache:
        _cache["nc"] = _build()
    return _cache["nc"]


def _prep_inputs(x_seq, w1, w2):
    """Host-side transpose/split/quantize. Returns per-core in_maps."""
    import ml_dtypes

    F8 = ml_dtypes.float8_e4m3

    x_seq = np.ascontiguousarray(x_seq, dtype=np.float32)
    w1 = np.ascontiguousarray(w1, dtype=np.float32)
    w2 = np.ascontiguousarray(w2, dtype=np.float32)

    wh = w1.astype(np.float16)                     # [HID, IN]
    wls = ((w1 - wh.astype(np.float32)) * SC)      # [HID, IN] fp32
    # packed fp16 main stationary [CH, NCH*HID]
    wht = np.ascontiguousarray(
        wh.T.reshape(NCH, CH, HID).transpose(1, 0, 2).reshape(CH, NCH * HID)
    )
    # fp8 stationaries padded to MP cols
    wh8 = wh.astype(np.float32).astype(F8)         # fp8e4(wh16)
    wls8 = wls.astype(F8)

    def _pack_w8(wq):
        # [HID, IN] fp8 -> [CH, NCH, MP] (pad HID->MP with zeros)
        wt = np.zeros((CH, NCH, MP), F8)
        wt[:, :, 0:HID] = wq.T.reshape(NCH, CH, HID).transpose(1, 0, 2)
        return np.ascontiguousarray(wt.reshape(CH, NCH * MP))

    w2a = np.concatenate(
        [w2.T.astype(np.float32), -np.eye(OUT, dtype=np.float32)], axis=0
    )                                              # [HID+OUT, OUT]

    xh_full = x_seq.astype(np.float16)             # [T, B, IN]
    xl8_full = ((x_seq - xh_full.astype(np.float32)) * SC).astype(F8)

    def _pack_x(xc):
        # [TB, IN] -> x^T [IN, TB] -> [CH, ITERS*NCH*TILE]: partition p of
        # tile i holds chunks c=0..6 contiguously (7KB/3.5KB DMA descriptors)
        xt = xc.T.reshape(NCH, CH, ITERS, TILE)
        return np.ascontiguousarray(
            xt.transpose(1, 2, 0, 3).reshape(CH, ITERS * NCH * TILE)
        )

    wh8p = _pack_w8(wh8)
    wls8p = _pack_w8(wls8)

    in_maps = []
    for c in range(NCORES):
        xh_c = xh_full[:, c * BS : (c + 1) * BS, :].reshape(TB, IN)
        xl_c = xl8_full[:, c * BS : (c + 1) * BS, :].reshape(TB, IN)
        in_maps.append(
            {
                "xh": _pack_x(xh_c),
                "xl8": _pack_x(xl_c),
                "wht": wht,
                "wh8p": wh8p,
                "wls8p": wls8p,
                "w2a": w2a,
            }
        )
    return in_maps


def kernel(x_seq: np.ndarray, w1: np.ndarray, w2: np.ndarray) -> np.ndarray:
    from concourse.bass_utils import run_bass_kernel_spmd

    nc = _get_nc()
    in_maps = _prep_inputs(x_seq, w1, w2)
    try:
        res = run_bass_kernel_spmd(nc, in_maps, core_ids=list(range(NCORES)))
    except Exception:
        res = run_bass_kernel_spmd(nc, in_maps, core_ids=list(range(NCORES)))
    _cache["last_results"] = res

    full = np.empty((B, OUT), dtype=np.float32)
    for c in range(NCORES):
        full[c * BS : (c + 1) * BS, :] = res.results[c]["out"].T
    return full


# revision 19
# speedup vs baseline: 1.2465x; 1.2465x over previous
"""Trainium2 Bass kernel for nn_EvoSNN (2-layer leaky-integrate-and-fire SNN).

V3 strategy (8 NeuronCores, data-parallel over batch, 256 rows per core):
  This is a memory-regime problem: the binding resource is DMA of x
  (fp32 = 80MB/core = 224us at 358GB/s). V3 ships x in 3 bytes/elem
  instead of 4 -> DMA roofline ~168us/core:
    xh  = fp16(x)                   [2B]  (packed [112, 50*7*512])
    xl8 = fp8e4m3((x - xh) * 2048)  [1B]  (same packing)
  Accuracy comes from a 3-stream matmul decomposition (CPU-simulated
  rel err 4.1e-3 vs the 2e-2 gate):
    psA = xh @ wh16                 7 fp16 matmuls   (512 cyc each)
    psB = xl8 @ wh8  (x residual)   3 DoubleRow fp8 pairs + 1 regular
        + x8  @ wls8 (w residual)   3 DoubleRow fp8 pairs + 1 regular
    cur = psA + psB / 2048
  where x8 = fp8e4(xh) is converted ON DEVICE (scalar engine chunks 0-3,
  gpsimd chunks 4-6; both HW-probed exact) and wls8 = fp8e4((w1-wh16)*2048)
  carries the fp16 weight-rounding correction. DoubleRow contracts TWO
  112-row chunks per instruction at 0.5 cyc/row; the chunked [112,7,512]
  tile layout is already a valid [Ki,Ko=2,dim] DR operand (stride 512%16==0;
  stationaries are padded to 112 cols so stride 112%16==0). HW probe:
  3xDR+1 group matches fp64 ref to 8.7e-5.
  PE per tile: 7*512 + 2*(3*256+512) ~= 6100 cyc -> ~130us/core + layer2,
  under the 168us DMA roof; vector work (converts ~65us/engine + LIF DVE
  ~65us) also hides under DMA.
  Phase 2 (sequential LIF on DVE) and layer 2 (w2 fp32r matmul with -I
  fold for the spk2 reset) are inherited from V2, as is the LAG/SKEW
  software pipelining of phase-1 tiles vs phase-2 steps.
  V2 (4B exact fp16 limbs, 21 fp16 matmuls) measured 303us; V3 targets
  ~175-195us (DMA-bound).
"""

import sys

for _p in ("/opt/trn_rl_repo", "/root/.axon_site/_ro/trn_rl_repo"):
    if _p not in sys.path:
        sys.path.append(_p)

import numpy as np

T, B, IN, HID, OUT = 100, 2048, 784, 100, 10
NCORES = 8
BS = B // NCORES          # 256 batch rows per core
TB = T * BS               # 25600
TILE = 512                # tb columns per phase-1 tile (2 timesteps)
ITERS = TB // TILE        # 50
NCH = 7                   # K chunks of 112 over IN=784
CH = IN // NCH            # 112
MP = 112                  # padded stationary cols for fp8 DR (112%16==0)
LAG = 1
SKEW = 2                  # layer-1 runs SKEW steps ahead of layer-2
REPEAT = 1
SC = 2048.0               # residual limb scale
X_BUFS = 3
X8_BUFS = 3
PSA_BUFS = 3
PSB_BUFS = 3
PM2_BUFS = 2
SCALAR_CHUNKS = 7         # all chunks converted on scalar engine (its queue
                          # holds nothing else, so converts run ahead freely)
VARIANT = "full"          # full | dma | p1 | nodr | noconv  (ablation)
MERGE_ENGINE = "none"     # pool | vector | none (psB fold placement)

_cache = {}


def _build():
    import concourse.bacc as bacc
    import concourse.mybir as mybir
    from concourse.tile import TileContext

    F32 = mybir.dt.float32
    F32R = mybir.dt.float32r
    F16 = mybir.dt.float16
    F8E4 = mybir.dt.float8e4
    AO = mybir.AluOpType
    DR = mybir.MatmulPerfMode.DoubleRow
    COPY = mybir.ActivationFunctionType.Copy

    nc = bacc.Bacc("TRN2", target_bir_lowering=False, debug=False)
    xh = nc.dram_tensor("xh", [CH, ITERS * NCH * TILE], F16,
                        kind="ExternalInput").ap()
    xl8 = nc.dram_tensor("xl8", [CH, ITERS * NCH * TILE], F8E4,
                         kind="ExternalInput").ap()
    wht = nc.dram_tensor("wht", [CH, NCH * HID], F16,
                         kind="ExternalInput").ap()
    wh8p = nc.dram_tensor("wh8p", [CH, NCH * MP], F8E4,
                          kind="ExternalInput").ap()
    wls8p = nc.dram_tensor("wls8p", [CH, NCH * MP], F8E4,
                           kind="ExternalInput").ap()
    w2a = nc.dram_tensor("w2a", [HID, OUT], F32, kind="ExternalInput").ap()
    out = nc.dram_tensor("out", [OUT, BS], F32, kind="ExternalOutput").ap()

    with TileContext(nc) as tc:
        with (
            tc.tile_pool(name="const", bufs=1) as constp,
            tc.tile_pool(name="xt", bufs=X_BUFS) as xtp,
            tc.tile_pool(name="x8", bufs=X8_BUFS) as x8p,
            tc.tile_pool(name="st", bufs=1) as stp,
            tc.tile_pool(name="psa", bufs=PSA_BUFS, space="PSUM") as psap,
            tc.tile_pool(name="psb", bufs=PSB_BUFS, space="PSUM") as psbp,
            tc.tile_pool(name="pm2", bufs=PM2_BUFS, space="PSUM") as pm2p,
        ):
            # ---------------- weights ----------------
            wh = constp.tile([CH, NCH, HID], F16, tag="wh")
            nc.sync.dma_start(wh[:], wht.rearrange("p (c h) -> p c h", c=NCH))
            wh8 = constp.tile([CH, NCH, MP], F8E4, tag="wh8")
            nc.sync.dma_start(
                wh8[:], wh8p.rearrange("p (c h) -> p c h", c=NCH)
            )
            wls8 = constp.tile([CH, NCH, MP], F8E4, tag="wls8")
            nc.sync.dma_start(
                wls8[:], wls8p.rearrange("p (c h) -> p c h", c=NCH)
            )

            w2f = constp.tile([HID, OUT], F32, tag="w2f")
            nc.sync.dma_start(w2f[:], w2a)
            w2r = constp.tile([HID, OUT], F32R, tag="w2r")
            nc.sync.dma_start(w2r[:], w2f[:].bitcast(F32R))

            # ---------------- state ----------------
            mem1 = stp.tile([HID, BS], F32, tag="mem1")
            mem2 = stp.tile([OUT, BS], F32, tag="mem2")
            acc = stp.tile([OUT, BS], F32, tag="acc")
            nbuf = SKEW + 1
            spk1_tiles = []
            for k in range(nbuf):
                spk1_k = stp.tile([HID, BS], F32R, tag=f"spk1_{k}",
                                  name=f"spk1_{k}")
                spk1_tiles.append(spk1_k)
            spk2t = stp.tile([OUT, BS], F32R, tag="spk2")
            nc.gpsimd.memset(mem1[:], 0.0)
            nc.gpsimd.memset(mem2[:], 0.0)
            nc.gpsimd.memset(acc[:], 0.0)
            for k in range(nbuf):
                nc.gpsimd.memset(spk1_tiles[k][:].bitcast(F32), 0.0)
            nc.gpsimd.memset(spk2t[:].bitcast(F32), 0.0)
            # spk1 rotating buffers: step t writes spk1_bufs[t % (SKEW+1)];
            # layer-1 runs SKEW steps ahead of layer-2, so layer-2's matmul
            # still sees step t's spikes after later layer-1 steps completed
            spk1_bufs = [t[:] for t in spk1_tiles]
            spk2 = spk2t[:]
            spk2_f = spk2.bitcast(F32)
            # negI [OUT, OUT] f32r: folds "- spk2_prev" into the p2 PSUM group
            negI_f = constp.tile([OUT, OUT], F32, tag="negIf")
            nc.gpsimd.memset(negI_f[:], 0.0)
            nc.gpsimd.affine_select(
                out=negI_f[:], in_=negI_f[:], compare_op=AO.not_equal,
                fill=-1.0, base=0, pattern=[[-1, OUT]], channel_multiplier=1,
            )
            negI = constp.tile([OUT, OUT], F32R, tag="negIr")
            nc.sync.dma_start(negI[:], negI_f[:].bitcast(F32R))

            cur_tiles = []

            def phase1_iter(i):
                # packed layout: one 7KB-contiguous read per partition/tile
                span = NCH * TILE
                xh_t = xtp.tile([CH, NCH, TILE], F16, tag="xh")
                nc.sync.dma_start(
                    xh_t[:],
                    xh[:, span * i : span * (i + 1)].rearrange(
                        "p (c n) -> p c n", c=NCH
                    ),
                )
                xl_t = xtp.tile([CH, NCH, TILE], F8E4, tag="xl")
                nc.sync.dma_start(
                    xl_t[:],
                    xl8[:, span * i : span * (i + 1)].rearrange(
                        "p (c n) -> p c n", c=NCH
                    ),
                )
                if VARIANT == "dma":
                    return
                # x8 = fp8e4(xh), split across scalar + gpsimd engines
                if VARIANT not in ("noconv", "p1noconv"):
                    x8_t = x8p.tile([CH, NCH, TILE], F8E4, tag="x8")
                    nc.scalar.activation(
                        out=x8_t[:, 0:SCALAR_CHUNKS, :],
                        in_=xh_t[:, 0:SCALAR_CHUNKS, :], func=COPY,
                    )
                    if SCALAR_CHUNKS < NCH:
                        nc.gpsimd.tensor_copy(
                            out=x8_t[:, SCALAR_CHUNKS:NCH, :],
                            in_=xh_t[:, SCALAR_CHUNKS:NCH, :],
                        )
                else:
                    x8_t = xl_t  # timing-only stand-in
                # main stream: 7 fp16 matmuls
                psA = psap.tile([HID, TILE], F32, tag="psa")
                for c in range(NCH):
                    nc.tensor.matmul(
                        psA[:], wh[:, c, :], xh_t[:, c, :],
                        start=(c == 0), stop=(c == NCH - 1),
                    )
                if VARIANT == "p1main":
                    cur_tiles.append((psA, psA))
                    return
                # correction stream: x-res (xl8 @ wh8) + w-res (x8 @ wls8),
                # 3 DoubleRow pairs + 1 regular each, one PSUM group
                psB = psbp.tile([MP, TILE], F32, tag="psb")
                if VARIANT in ("nodr", "p1nodr"):
                    for c in range(NCH):
                        nc.tensor.matmul(
                            psB[0:HID, :], wh8[:, c, 0:HID], xl_t[:, c, :],
                            start=(c == 0), stop=False,
                        )
                    for c in range(NCH):
                        nc.tensor.matmul(
                            psB[0:HID, :], wls8[:, c, 0:HID], x8_t[:, c, :],
                            start=False, stop=(c == NCH - 1),
                        )
                else:
                    for j in range(3):
                        nc.tensor.matmul(
                            psB[:], wh8[:, 2 * j : 2 * j + 2, :],
                            xl_t[:, 2 * j : 2 * j + 2, :],
                            start=(j == 0), stop=False, perf_mode=DR,
                        )
                    nc.tensor.matmul(
                        psB[0:HID, :], wh8[:, NCH - 1, 0:HID],
                        xl_t[:, NCH - 1, :], start=False, stop=False,
                    )
                    for j in range(3):
                        nc.tensor.matmul(
                            psB[:], wls8[:, 2 * j : 2 * j + 2, :],
                            x8_t[:, 2 * j : 2 * j + 2, :],
                            start=False, stop=False, perf_mode=DR,
                        )
                    nc.tensor.matmul(
                        psB[0:HID, :], wls8[:, NCH - 1, 0:HID],
                        x8_t[:, NCH - 1, :], start=False, stop=True,
                    )
                # fold the correction into psA once per tile: one [100,512]
                # op replaces a per-step DVE stt
                if MERGE_ENGINE == "vector":
                    nc.vector.scalar_tensor_tensor(
                        out=psA[:], in0=psB[0:HID, :], scalar=1.0 / SC,
                        in1=psA[:], op0=AO.mult, op1=AO.add,
                    )
                cur_tiles.append((psA, psB))

            def layer1_step(t):
                i, half = divmod(t, 2)
                sl = slice(BS * half, BS * (half + 1))
                psA, psB = cur_tiles[i]
                nc.vector.scalar_tensor_tensor(
                    out=mem1[:], in0=mem1[:], scalar=0.9, in1=psA[:, sl],
                    op0=AO.mult, op1=AO.add,
                )
                if MERGE_ENGINE == "none":
                    nc.vector.scalar_tensor_tensor(
                        out=mem1[:], in0=psB[0:HID, sl], scalar=1.0 / SC,
                        in1=mem1[:], op0=AO.mult, op1=AO.add,
                    )
                nc.vector.tensor_tensor(
                    out=mem1[:], in0=mem1[:],
                    in1=spk1_bufs[(t - 1) % nbuf].bitcast(F32), op=AO.subtract
                )
                nc.vector.tensor_scalar(
                    out=spk1_bufs[t % nbuf], in0=mem1[:], scalar1=1.0,
                    scalar2=None, op0=AO.is_gt,
                )

            def layer2_step(t):
                # DVE updates mem2 (PSUM read; Pool can't touch PSUM);
                # Pool thresholds + accumulates (SBUF only)
                p2 = pm2p.tile([OUT, BS], F32, tag="p2")
                nc.tensor.matmul(
                    p2[:], w2r[:], spk1_bufs[t % nbuf], start=True, stop=False
                )
                nc.tensor.matmul(p2[:], negI[:], spk2, start=False, stop=True)
                nc.vector.scalar_tensor_tensor(
                    out=mem2[:], in0=mem2[:], scalar=0.9, in1=p2[:],
                    op0=AO.mult, op1=AO.add,
                )
                nc.gpsimd.tensor_scalar(
                    out=spk2, in0=mem2[:], scalar1=1.0, scalar2=None,
                    op0=AO.is_gt,
                )
                nc.gpsimd.tensor_tensor(
                    out=acc[:], in0=acc[:], in1=spk2_f, op=AO.add
                )

            def phase2_pair(ta, tb_):
                # layer-1 runs SKEW steps ahead of layer-2 so the DVE never
                # waits on layer-2's PE matmul (its sem round-trip hides
                # behind SKEW steps of layer-1 work).
                for t in (ta, tb_):
                    if t < T:
                        layer1_step(t)
                    if 0 <= t - SKEW < T:
                        layer2_step(t - SKEW)

            run_p2 = VARIANT not in (
                "dma", "p1", "p1nodr", "p1noconv", "p1main"
            )
            for _rep in range(REPEAT):
                cur_tiles.clear()
                for i in range(ITERS):
                    if run_p2 and i >= LAG:
                        phase2_pair(2 * (i - LAG), 2 * (i - LAG) + 1)
                    phase1_iter(i)
                if run_p2:
                    t0 = 2 * (ITERS - LAG)
                    for t in range(t0, T + SKEW + 1, 2):
                        phase2_pair(t, t + 1)

            nc.sync.dma_start(out, acc[:])

    nc.compile()
    return nc


def _get_nc():
    if "nc" not in _cache:
        _cache["nc"] = _build()
    return _cache["nc"]


def _prep_inputs(x_seq, w1, w2):
    """Host-side transpose/split/quantize. Returns per-core in_maps."""
    import ml_dtypes

    F8 = ml_dtypes.float8_e4m3

    x_seq = np.ascontiguousarray(x_seq, dtype=np.float32)
    w1 = np.ascontiguousarray(w1, dtype=np.float32)
    w2 = np.ascontiguousarray(w2, dtype=np.float32)

    wh = w1.astype(np.float16)                     # [HID, IN]
    wls = ((w1 - wh.astype(np.float32)) * SC)      # [HID, IN] fp32
    # packed fp16 main stationary [CH, NCH*HID]
    wht = np.ascontiguousarray(
        wh.T.reshape(NCH, CH, HID).transpose(1, 0, 2).reshape(CH, NCH * HID)
    )
    # fp8 stationaries padded to MP cols
    wh8 = wh.astype(np.float32).astype(F8)         # fp8e4(wh16)
    wls8 = wls.astype(F8)

    def _pack_w8(wq):
        # [HID, IN] fp8 -> [CH, NCH, MP] (pad HID->MP with zeros)
        wt = np.zeros((CH, NCH, MP), F8)
        wt[:, :, 0:HID] = wq.T.reshape(NCH, CH, HID).transpose(1, 0, 2)
        return np.ascontiguousarray(wt.reshape(CH, NCH * MP))

    w2a = np.concatenate(
        [w2.T.astype(np.float32), -np.eye(OUT, dtype=np.float32)], axis=0
    )                                              # [HID+OUT, OUT]

    xh_full = x_seq.astype(np.float16)             # [T, B, IN]
    xl8_full = ((x_seq - xh_full.astype(np.float32)) * SC).astype(F8)

    def _pack_x(xc):
        # [TB, IN] -> x^T [IN, TB] -> [CH, ITERS*NCH*TILE]: partition p of
        # tile i holds chunks c=0..6 contiguously (7KB/3.5KB DMA descriptors)
        xt = xc.T.reshape(NCH, CH, ITERS, TILE)
        return np.ascontiguousarray(
            xt.transpose(1, 2, 0, 3).reshape(CH, ITERS * NCH * TILE)
        )

    wh8p = _pack_w8(wh8)
    wls8p = _pack_w8(wls8)

    in_maps = []
    for c in range(NCORES):
        xh_c = xh_full[:, c * BS : (c + 1) * BS, :].reshape(TB, IN)
        xl_c = xl8_full[:, c * BS : (c + 1) * BS, :].reshape(TB, IN)
        in_maps.append(
            {
                "xh": _pack_x(xh_c),
                "xl8": _pack_x(xl_c),
                "wht": wht,
                "wh8p": wh8p,
                "wls8p": wls8p,
                "w2a": w2a,
            }
        )
    return in_maps


def kernel(x_seq: np.ndarray, w1: np.ndarray, w2: np.ndarray) -> np.ndarray:
    from concourse.bass_utils import run_bass_kernel_spmd

    nc = _get_nc()
    in_maps = _prep_inputs(x_seq, w1, w2)
    try:
        res = run_bass_kernel_spmd(nc, in_maps, core_ids=list(range(NCORES)))
    except Exception:
        res = run_bass_kernel_spmd(nc, in_maps, core_ids=list(range(NCORES)))
    _cache["last_results"] = res

    full = np.empty((B, OUT), dtype=np.float32)
    for c in range(NCORES):
        full[c * BS : (c + 1) * BS, :] = res.results[c]["out"].T
    return full


# revision 25
# speedup vs baseline: 3.9500x; 3.1688x over previous
"""Trainium2 Bass kernel for nn_EvoSNN (2-layer leaky-integrate-and-fire SNN).

V3 strategy (8 NeuronCores, data-parallel over batch, 256 rows per core):
  This is a memory-regime problem: the binding resource is DMA of x
  (fp32 = 80MB/core = 224us at 358GB/s). V3 ships x in 3 bytes/elem
  instead of 4 -> DMA roofline ~168us/core:
    xh  = fp16(x)                   [2B]  (packed [112, 50*7*512])
    xl8 = fp8e4m3((x - xh) * 2048)  [1B]  (same packing)
  Accuracy comes from a 3-stream matmul decomposition (CPU-simulated
  rel err 4.1e-3 vs the 2e-2 gate):
    psA = xh @ wh16                 7 fp16 matmuls   (512 cyc each)
    psB = xl8 @ wh8  (x residual)   3 DoubleRow fp8 pairs + 1 regular
        + x8  @ wls8 (w residual)   3 DoubleRow fp8 pairs + 1 regular
    cur = psA + psB / 2048
  where x8 = fp8e4(xh) is converted ON DEVICE (scalar engine chunks 0-3,
  gpsimd chunks 4-6; both HW-probed exact) and wls8 = fp8e4((w1-wh16)*2048)
  carries the fp16 weight-rounding correction. DoubleRow contracts TWO
  112-row chunks per instruction at 0.5 cyc/row; the chunked [112,7,512]
  tile layout is already a valid [Ki,Ko=2,dim] DR operand (stride 512%16==0;
  stationaries are padded to 112 cols so stride 112%16==0). HW probe:
  3xDR+1 group matches fp64 ref to 8.7e-5.
  PE per tile: 7*512 + 2*(3*256+512) ~= 6100 cyc -> ~130us/core + layer2,
  under the 168us DMA roof; vector work (converts ~65us/engine + LIF DVE
  ~65us) also hides under DMA.
  Phase 2 (sequential LIF on DVE) and layer 2 (w2 fp32r matmul with -I
  fold for the spk2 reset) are inherited from V2, as is the LAG/SKEW
  software pipelining of phase-1 tiles vs phase-2 steps.
  V2 (4B exact fp16 limbs, 21 fp16 matmuls) measured 303us; V3 targets
  ~175-195us (DMA-bound).
"""

import sys

for _p in ("/opt/trn_rl_repo", "/root/.axon_site/_ro/trn_rl_repo"):
    if _p not in sys.path:
        sys.path.append(_p)

import numpy as np

T, B, IN, HID, OUT = 100, 2048, 784, 100, 10
NCORES = 8
BS = B // NCORES          # 256 batch rows per core
TB = T * BS               # 25600
TILE = 512                # tb columns per phase-1 tile (2 timesteps)
ITERS = TB // TILE        # 50
NCH = 7                   # K chunks of 112 over IN=784
CH = IN // NCH            # 112
MP = 112                  # padded stationary cols for fp8 DR (112%16==0)
LAG = 2
SKEW = 2                  # layer-1 runs SKEW steps ahead of layer-2
REPEAT = 1
SC = 2048.0               # residual limb scale
X_BUFS = 3
X8_BUFS = 3
PSA_BUFS = 3
PSB_BUFS = 3
PM2_BUFS = 2
SCALAR_CHUNKS = 7         # all chunks converted on scalar engine (its queue
                          # holds nothing else, so converts run ahead freely)
VARIANT = "full"          # full | dma | p1 | nodr | noconv  (ablation)
MERGE_ENGINE = "none"     # "none": psB folded per-step on DVE (a DVE stt may
                          # touch at most ONE PSUM operand, so a single
                          # psA+=psB/SC merge op is not encodable)

_cache = {}


def _build():
    import concourse.bacc as bacc
    import concourse.mybir as mybir
    from concourse.tile import TileContext

    F32 = mybir.dt.float32
    F32R = mybir.dt.float32r
    F16 = mybir.dt.float16
    F8E4 = mybir.dt.float8e4
    AO = mybir.AluOpType
    DR = mybir.MatmulPerfMode.DoubleRow
    COPY = mybir.ActivationFunctionType.Copy

    nc = bacc.Bacc("TRN2", target_bir_lowering=False, debug=False)
    xh = nc.dram_tensor("xh", [CH, ITERS * NCH * TILE], F16,
                        kind="ExternalInput").ap()
    xl8 = nc.dram_tensor("xl8", [CH, ITERS * NCH * TILE], F8E4,
                         kind="ExternalInput").ap()
    wht = nc.dram_tensor("wht", [CH, NCH * HID], F16,
                         kind="ExternalInput").ap()
    wh8p = nc.dram_tensor("wh8p", [CH, NCH * MP], F8E4,
                          kind="ExternalInput").ap()
    wls8p = nc.dram_tensor("wls8p", [CH, NCH * MP], F8E4,
                           kind="ExternalInput").ap()
    w2a = nc.dram_tensor("w2a", [HID, OUT], F32, kind="ExternalInput").ap()
    out = nc.dram_tensor("out", [OUT, BS], F32, kind="ExternalOutput").ap()

    with TileContext(nc) as tc:
        with (
            tc.tile_pool(name="const", bufs=1) as constp,
            tc.tile_pool(name="xt", bufs=X_BUFS) as xtp,
            tc.tile_pool(name="x8", bufs=X8_BUFS) as x8p,
            tc.tile_pool(name="st", bufs=1) as stp,
            tc.tile_pool(name="psa", bufs=PSA_BUFS, space="PSUM") as psap,
            tc.tile_pool(name="psb", bufs=PSB_BUFS, space="PSUM") as psbp,
            tc.tile_pool(name="pm2", bufs=PM2_BUFS, space="PSUM") as pm2p,
        ):
            # ---------------- weights ----------------
            wh = constp.tile([CH, NCH, HID], F16, tag="wh")
            nc.sync.dma_start(wh[:], wht.rearrange("p (c h) -> p c h", c=NCH))
            wh8 = constp.tile([CH, NCH, MP], F8E4, tag="wh8")
            nc.sync.dma_start(
                wh8[:], wh8p.rearrange("p (c h) -> p c h", c=NCH)
            )
            wls8 = constp.tile([CH, NCH, MP], F8E4, tag="wls8")
            nc.sync.dma_start(
                wls8[:], wls8p.rearrange("p (c h) -> p c h", c=NCH)
            )

            w2f = constp.tile([HID, OUT], F32, tag="w2f")
            nc.sync.dma_start(w2f[:], w2a)
            w2r = constp.tile([HID, OUT], F32R, tag="w2r")
            nc.sync.dma_start(w2r[:], w2f[:].bitcast(F32R))

            # ---------------- state ----------------
            mem1 = stp.tile([HID, BS], F32, tag="mem1")
            mem2 = stp.tile([OUT, BS], F32, tag="mem2")
            acc = stp.tile([OUT, BS], F32, tag="acc")
            nbuf = SKEW + 1
            spk1_tiles = []
            for k in range(nbuf):
                spk1_k = stp.tile([HID, BS], F32R, tag=f"spk1_{k}",
                                  name=f"spk1_{k}")
                spk1_tiles.append(spk1_k)
            spk2t = stp.tile([OUT, BS], F32R, tag="spk2")
            nc.gpsimd.memset(mem1[:], 0.0)
            nc.gpsimd.memset(mem2[:], 0.0)
            nc.gpsimd.memset(acc[:], 0.0)
            for k in range(nbuf):
                nc.gpsimd.memset(spk1_tiles[k][:].bitcast(F32), 0.0)
            nc.gpsimd.memset(spk2t[:].bitcast(F32), 0.0)
            # spk1 rotating buffers: step t writes spk1_bufs[t % (SKEW+1)];
            # layer-1 runs SKEW steps ahead of layer-2, so layer-2's matmul
            # still sees step t's spikes after later layer-1 steps completed
            spk1_bufs = [t[:] for t in spk1_tiles]
            spk2 = spk2t[:]
            spk2_f = spk2.bitcast(F32)
            # negI [OUT, OUT] f32r: folds "- spk2_prev" into the p2 PSUM group
            negI_f = constp.tile([OUT, OUT], F32, tag="negIf")
            nc.gpsimd.memset(negI_f[:], 0.0)
            nc.gpsimd.affine_select(
                out=negI_f[:], in_=negI_f[:], compare_op=AO.not_equal,
                fill=-1.0, base=0, pattern=[[-1, OUT]], channel_multiplier=1,
            )
            negI = constp.tile([OUT, OUT], F32R, tag="negIr")
            nc.sync.dma_start(negI[:], negI_f[:].bitcast(F32R))

            cur_tiles = []
            pend = {}

            def phase1_main(i):
                # packed layout: one 7KB-contiguous read per partition/tile
                span = NCH * TILE
                xh_t = xtp.tile([CH, NCH, TILE], F16, tag="xh")
                nc.sync.dma_start(
                    xh_t[:],
                    xh[:, span * i : span * (i + 1)].rearrange(
                        "p (c n) -> p c n", c=NCH
                    ),
                )
                xl_t = xtp.tile([CH, NCH, TILE], F8E4, tag="xl")
                nc.sync.dma_start(
                    xl_t[:],
                    xl8[:, span * i : span * (i + 1)].rearrange(
                        "p (c n) -> p c n", c=NCH
                    ),
                )
                if VARIANT == "dma":
                    return
                # x8 = fp8e4(xh) on the scalar engine (sole scalar-queue user)
                if VARIANT not in ("noconv", "p1noconv"):
                    x8_t = x8p.tile([CH, NCH, TILE], F8E4, tag="x8")
                    nc.scalar.activation(
                        out=x8_t[:], in_=xh_t[:], func=COPY,
                    )
                else:
                    x8_t = xl_t  # timing-only stand-in
                # main stream: 7 fp16 matmuls
                psA = psap.tile([HID, TILE], F32, tag="psa")
                for c in range(NCH):
                    nc.tensor.matmul(
                        psA[:], wh[:, c, :], xh_t[:, c, :],
                        start=(c == 0), stop=(c == NCH - 1),
                    )
                if VARIANT == "p1main":
                    cur_tiles.append((psA, psA))
                    return
                pend[i] = (psA, xl_t, x8_t)

            def phase1_corr(i):
                # psB + merge are emitted one phase-1 slot late so the PE
                # queue never stalls on the scalar convert of the same tile
                if VARIANT in ("dma", "p1main") or i not in pend:
                    return
                psA, xl_t, x8_t = pend.pop(i)
                psB = psbp.tile([MP, TILE], F32, tag="psb")
                if VARIANT in ("nodr", "p1nodr"):
                    for c in range(NCH):
                        nc.tensor.matmul(
                            psB[0:HID, :], wh8[:, c, 0:HID], xl_t[:, c, :],
                            start=(c == 0), stop=False,
                        )
                    for c in range(NCH):
                        nc.tensor.matmul(
                            psB[0:HID, :], wls8[:, c, 0:HID], x8_t[:, c, :],
                            start=False, stop=(c == NCH - 1),
                        )
                else:
                    for j in range(3):
                        nc.tensor.matmul(
                            psB[:], wh8[:, 2 * j : 2 * j + 2, :],
                            xl_t[:, 2 * j : 2 * j + 2, :],
                            start=(j == 0), stop=False, perf_mode=DR,
                        )
                    nc.tensor.matmul(
                        psB[0:HID, :], wh8[:, NCH - 1, 0:HID],
                        xl_t[:, NCH - 1, :], start=False, stop=False,
                    )
                    for j in range(3):
                        nc.tensor.matmul(
                            psB[:], wls8[:, 2 * j : 2 * j + 2, :],
                            x8_t[:, 2 * j : 2 * j + 2, :],
                            start=False, stop=False, perf_mode=DR,
                        )
                    nc.tensor.matmul(
                        psB[0:HID, :], wls8[:, NCH - 1, 0:HID],
                        x8_t[:, NCH - 1, :], start=False, stop=True,
                    )
                # fold the correction into psA once per tile: one [100,512]
                # op replaces a per-step DVE stt
                if MERGE_ENGINE == "vector":
                    nc.vector.scalar_tensor_tensor(
                        out=psA[:], in0=psB[0:HID, :], scalar=1.0 / SC,
                        in1=psA[:], op0=AO.mult, op1=AO.add,
                    )
                cur_tiles.append((psA, psB))

            def layer1_step(t):
                i, half = divmod(t, 2)
                sl = slice(BS * half, BS * (half + 1))
                psA, psB = cur_tiles[i]
                nc.vector.scalar_tensor_tensor(
                    out=mem1[:], in0=mem1[:], scalar=0.9, in1=psA[:, sl],
                    op0=AO.mult, op1=AO.add,
                )
                if MERGE_ENGINE == "none":
                    nc.vector.scalar_tensor_tensor(
                        out=mem1[:], in0=psB[0:HID, sl], scalar=1.0 / SC,
                        in1=mem1[:], op0=AO.mult, op1=AO.add,
                    )
                nc.vector.tensor_tensor(
                    out=mem1[:], in0=mem1[:],
                    in1=spk1_bufs[(t - 1) % nbuf].bitcast(F32), op=AO.subtract
                )
                nc.vector.tensor_scalar(
                    out=spk1_bufs[t % nbuf], in0=mem1[:], scalar1=1.0,
                    scalar2=None, op0=AO.is_gt,
                )

            def layer2_step(t):
                # DVE updates mem2 (PSUM read; Pool can't touch PSUM);
                # Pool thresholds + accumulates (SBUF only)
                p2 = pm2p.tile([OUT, BS], F32, tag="p2")
                nc.tensor.matmul(
                    p2[:], w2r[:], spk1_bufs[t % nbuf], start=True, stop=False
                )
                nc.tensor.matmul(p2[:], negI[:], spk2, start=False, stop=True)
                nc.vector.scalar_tensor_tensor(
                    out=mem2[:], in0=mem2[:], scalar=0.9, in1=p2[:],
                    op0=AO.mult, op1=AO.add,
                )
                nc.vector.tensor_scalar(
                    out=spk2, in0=mem2[:], scalar1=1.0, scalar2=None,
                    op0=AO.is_gt,
                )
                nc.gpsimd.tensor_tensor(
                    out=acc[:], in0=acc[:], in1=spk2_f, op=AO.add
                )

            def phase2_pair(ta, tb_):
                # layer-1 runs SKEW steps ahead of layer-2 so the DVE never
                # waits on layer-2's PE matmul (its sem round-trip hides
                # behind SKEW steps of layer-1 work).
                for t in (ta, tb_):
                    if t < T:
                        layer1_step(t)
                    if 0 <= t - SKEW < T:
                        layer2_step(t - SKEW)

            run_p2 = VARIANT not in (
                "dma", "p1", "p1nodr", "p1noconv", "p1main"
            )
            for _rep in range(REPEAT):
                cur_tiles.clear()
                pend.clear()
                for i in range(ITERS):
                    if run_p2 and i >= LAG:
                        phase2_pair(2 * (i - LAG), 2 * (i - LAG) + 1)
                    phase1_main(i)
                    phase1_corr(i - 1)
                phase1_corr(ITERS - 1)
                if run_p2:
                    t0 = 2 * (ITERS - LAG)
                    for t in range(t0, T + SKEW + 1, 2):
                        phase2_pair(t, t + 1)

            nc.sync.dma_start(out, acc[:])

    nc.compile()
    return nc


def _get_nc():
    if "nc" not in _cache:
        _cache["nc"] = _build()
    return _cache["nc"]


def _prep_inputs(x_seq, w1, w2):
    """Host-side transpose/split/quantize. Returns per-core in_maps."""
    import ml_dtypes

    F8 = ml_dtypes.float8_e4m3

    x_seq = np.ascontiguousarray(x_seq, dtype=np.float32)
    w1 = np.ascontiguousarray(w1, dtype=np.float32)
    w2 = np.ascontiguousarray(w2, dtype=np.float32)

    wh = w1.astype(np.float16)                     # [HID, IN]
    wls = ((w1 - wh.astype(np.float32)) * SC)      # [HID, IN] fp32
    # packed fp16 main stationary [CH, NCH*HID]
    wht = np.ascontiguousarray(
        wh.T.reshape(NCH, CH, HID).transpose(1, 0, 2).reshape(CH, NCH * HID)
    )
    # fp8 stationaries padded to MP cols
    wh8 = wh.astype(np.float32).astype(F8)         # fp8e4(wh16)
    wls8 = wls.astype(F8)

    def _pack_w8(wq):
        # [HID, IN] fp8 -> [CH, NCH, MP] (pad HID->MP with zeros)
        wt = np.zeros((CH, NCH, MP), F8)
        wt[:, :, 0:HID] = wq.T.reshape(NCH, CH, HID).transpose(1, 0, 2)
        return np.ascontiguousarray(wt.reshape(CH, NCH * MP))

    w2a = np.concatenate(
        [w2.T.astype(np.float32), -np.eye(OUT, dtype=np.float32)], axis=0
    )                                              # [HID+OUT, OUT]

    xh_full = x_seq.astype(np.float16)             # [T, B, IN]
    xl8_full = ((x_seq - xh_full.astype(np.float32)) * SC).astype(F8)

    def _pack_x(xc):
        # [TB, IN] -> x^T [IN, TB] -> [CH, ITERS*NCH*TILE]: partition p of
        # tile i holds chunks c=0..6 contiguously (7KB/3.5KB DMA descriptors)
        xt = xc.T.reshape(NCH, CH, ITERS, TILE)
        return np.ascontiguousarray(
            xt.transpose(1, 2, 0, 3).reshape(CH, ITERS * NCH * TILE)
        )

    wh8p = _pack_w8(wh8)
    wls8p = _pack_w8(wls8)

    in_maps = []
    for c in range(NCORES):
        xh_c = xh_full[:, c * BS : (c + 1) * BS, :].reshape(TB, IN)
        xl_c = xl8_full[:, c * BS : (c + 1) * BS, :].reshape(TB, IN)
        in_maps.append(
            {
                "xh": _pack_x(xh_c),
                "xl8": _pack_x(xl_c),
                "wht": wht,
                "wh8p": wh8p,
                "wls8p": wls8p,
                "w2a": w2a,
            }
        )
    return in_maps


def kernel(x_seq: np.ndarray, w1: np.ndarray, w2: np.ndarray) -> np.ndarray:
    from concourse.bass_utils import run_bass_kernel_spmd

    nc = _get_nc()
    in_maps = _prep_inputs(x_seq, w1, w2)
    try:
        res = run_bass_kernel_spmd(nc, in_maps, core_ids=list(range(NCORES)))
    except Exception:
        res = run_bass_kernel_spmd(nc, in_maps, core_ids=list(range(NCORES)))
    _cache["last_results"] = res

    full = np.empty((B, OUT), dtype=np.float32)
    for c in range(NCORES):
        full[c * BS : (c + 1) * BS, :] = res.results[c]["out"].T
    return full
